# revision 50
# baseline (speedup 1.0000x reference)
"""Causal full attention (B=2, L=2048, H=16, E=64) on 8 trn2 NeuronCores.

Sharding: 32 (b, h) head-slices split 4-per-core; each core runs the same
Bass program on its own slice (data parallel over batch*heads, per the
sharding hint), no cross-core comms.

Design (row-tiled QK^T, ACT+DVE split exp, host finalize):
  - Heads processed in PAIRS: Q^T/K^T stacked [128, L] f16 (head A rows
    0:64, head B rows 64:128). QK^T for the two heads runs as two
    concurrent row-tiled matmuls (tile_position (0,0)/(64,0) derived from
    base partitions), halving mm1 wall time vs contraction-64 alone.
  - Host pre-scales Q by scale/64, so PSUM scores arrive as s = x/64
    where x is the true scaled logit.
  - exp is the throughput wall (1 elem/lane/cycle on ScalarE), so it is
    split between ScalarE (exact exp(64*s - 2) via the free scale/bias)
    and a two-instruction custom VectorE pipeline: deg-4 Taylor of
    exp(x/16) then ^16 * e^-2 (the DVE fused-op budget is 8 ALU ops, so
    one instruction cannot reach exp accuracy; two passes match the
    ScalarE path to ~1e-4). Work is greedily load-balanced; the shared
    e^-2 factor cancels in the softmax division.
  - V is augmented with a ones column (65 rows); the P@V matmul then
    yields the softmax denominator for free in row 64.
  - Output O^T [65, 512] per (head, q-chunk) accumulates in PSUM over
    s-tiles, is copied to SBUF (halves split across ScalarE/VectorE),
    DMA'd out as [HPC, 65, L] f32; the host does denominator division and
    the [E, L] -> [L, E] transpose (cheap numpy, off the HW critical
    path).
  - PSUM: 3-ring of [128, 2, 512] score tiles (6 banks) + 2 per-head OT
    accumulators (2 banks) = all 8 banks.
  - Emission is batched 2-3 steps per stage so the PE stays in one tiling
    mode (64-row for QK^T vs 128 for PV) for several matmuls at a time -
    each mode switch drains the TensorE (~16us/iter when alternating
    every matmul).
  - DVE pass2 is emitted lazily so back-to-back DVE score tiles free the
    PSUM ring after pass1; OT evacuation copies are emitted ahead of exp
    calls on both engines so the next accumulation group is not stalled.
"""

import numpy as np

B, L, H, E = 2, 2048, 16, 64
N_CORES = 8
HPC = B * H // N_CORES   # head-slices per core = 4
NPAIR = HPC // 2         # head pairs per core = 2
P = 128
NT = L // P              # 16 s-tiles of 128
W = 512                  # q-chunk width (one PSUM bank)
NC_ = L // W             # 4 q-chunks
SCALE = 1.0 / np.sqrt(E)
EXP_BIAS = 2.0           # both engines compute exp(x - EXP_BIAS)
POW_N = 64.0             # host folds SCALE/POW_N into Q

_CACHE = {}


def _register_exp_ops():
    """Register a two-instruction VectorE exp pipeline (the DVE budget is 8
    ALU ops per fused instruction, so one instruction cannot reach exp
    accuracy over the score range):

      pass1: p = deg-4 Taylor of exp(x/16) in s = x/64:
             p = (((c*s + c)*s + b)*s + a)*s + 1, a=4, b=8, c=64/6
             (c3 == c4 == 64/6, so three scalar slots suffice) - 8 ops.
      pass2: out = (p^16) * e^-B via 4 squarings + one multiply - 5 ops.

    Combined: exp(x - B) to ~1e-4 relative over |x| <= 7, matching the
    ScalarE-exact path (both engines share the e^-B factor, which cancels
    in the softmax division)."""
    from concourse import dve_ops
    from concourse.dve_spec import (
        Spec,
        Src0,
        C0,
        C1,
        C2,
        One,
        sq,
        lower,
        _has_src1 as has_src1,
    )
    from concourse.dve_uop import DveOpSpec
    from concourse.dve_table_gen import dve_ver_for

    names = ("EXP_POLY16_ANT", "EXP_SQ4S_ANT")
    have = {op.name: op for op in dve_ops.OPS if op.name in names}
    if len(have) == 2:
        return have[names[0]], have[names[1]]

    body1 = (((C2 * Src0 + C2) * Src0 + C1) * Src0 + C0) * Src0 + One

    def ref1(in0, in1=None, s0=0.0, s1=0.0, imm2=0.0):
        s = in0.astype(np.float32)
        return (
            ((np.float32(imm2) * s + np.float32(imm2)) * s + np.float32(s1)) * s
            + np.float32(s0)
        ) * s + np.float32(1.0)

    b2 = Src0
    for _ in range(4):
        b2 = sq(b2)
    body2 = b2 * C0

    def ref2(in0, in1=None, s0=0.0, s1=0.0, imm2=0.0):
        r = in0.astype(np.float32)
        for _ in range(4):
            r = r * r
        return r * np.float32(s0)

    ver = dve_ver_for("TRN2")
    out = []
    for name, body, ref in (
        (names[0], body1, ref1),
        (names[1], body2, ref2),
    ):
        spec = Spec(body=body, reference=ref)
        row = dve_ops._CUSTOM_DVE_ROW_BASE + len(dve_ops.OPS)
        assert row < 0x20
        tmp = DveOpSpec(
            name=name, opcode=row, uops=lower(spec, ver=ver), rd1_en=has_src1(spec)
        )
        op = dve_ops.DveOp(name, spec, subdim=False, uops_sha={ver: tmp.sha(ver)})
        dve_ops.OPS.append(op)
        dve_ops.CUSTOM_DVE_SPECS[name] = spec
        dve_ops._SUB_OPCODE_FOR_NAME[name] = row
        out.append(op)
    return tuple(out)


def _build_nc(loop_iters=None, variant="full"):
    import concourse.bass as bass
    import concourse.tile as tile
    from concourse import mybir, bacc

    exp_p1, exp_p2 = _register_exp_ops()

    f32 = mybir.dt.float32
    f16 = mybir.dt.float16

    nc = bacc.Bacc("TRN2", target_bir_lowering=False, debug=False)

    q_d = nc.dram_tensor("q", [NPAIR, P, L], f16, kind="ExternalInput")
    k_d = nc.dram_tensor("k", [NPAIR, P, L], f16, kind="ExternalInput")
    v_d = nc.dram_tensor("v", [NPAIR, P, 2, NT, E + 1], f16, kind="ExternalInput")
    o_d = nc.dram_tensor("o", [HPC, E + 1, L], f32, kind="ExternalOutput")

    # per-(c,j) step list for one pair: q-chunk c covers q in [cW, (c+1)W);
    # s-tile j valid when j*P < (c+1)*W; within the chunk the valid q start
    # is qrel0 = max(0, j*P - c*W); diag when the 128-wide diagonal block
    # lands inside this chunk.
    pair_steps = []
    for c in range(NC_):
        jmax = ((c + 1) * W) // P  # exclusive
        for j in range(jmax):
            qrel0 = max(0, j * P - c * W)
            diag = j * P >= c * W
            first = j == 0
            last = j == jmax - 1
            pair_steps.append((c, j, qrel0, diag, first, last))
    NSTEP = len(pair_steps)  # 40

    # exp/copy engine load balancer (ns-scale costs from the cost model)
    bal = {"act": 0.0, "dve": 0.0}

    def pick_engine(cost_act, cost_dve):
        if bal["act"] + cost_act <= bal["dve"] + cost_dve:
            bal["act"] += cost_act
            return "act"
        bal["dve"] += cost_dve
        return "dve"

    with tile.TileContext(nc) as tc:
        with (
            tc.tile_pool(name="const", bufs=1) as const_pool,
            tc.tile_pool(name="io", bufs=2) as io_pool,
            tc.tile_pool(name="ex", bufs=8) as ex_pool,
            tc.tile_pool(name="oc", bufs=4) as oc_pool,
            tc.tile_pool(name="ps_sc", bufs=3, space="PSUM") as ps_sc,
            tc.tile_pool(name="ps_ot", bufs=1, space="PSUM") as ps_ot,
        ):
            bias_t = const_pool.tile([P, 1], f32)
            nc.vector.memset(bias_t[:, :], float(-EXP_BIAS))
            dummy_ex = None
            if variant in ("noexp",):
                dummy_ex = const_pool.tile([P, 2, W], f16)
                nc.vector.memset(dummy_ex[:, :, :], 0.001)
            # warm the exp table set in the preamble so the ~2.7us
            # ACT_TABLE_LOAD doesn't land inside the hardware loop body
            warm_t = const_pool.tile([P, 1], f32)
            nc.scalar.activation(
                warm_t[:, :], bias_t[:, :], mybir.ActivationFunctionType.Exp
            )
            import contextlib

            loop_cm = (
                tc.For_i(0, loop_iters, 1) if loop_iters else contextlib.nullcontext()
            )
            with loop_cm:
                qt_p, kt_p, v_p = {}, {}, {}

                def load_pair(p_, split=False):
                    qt = io_pool.tile([P, L], f16, tag=f"qt{p_ % 2}")
                    kt = io_pool.tile([P, L], f16, tag=f"kt{p_ % 2}")
                    va = io_pool.tile([P, 2, NT, E + 1], f16, tag=f"va{p_ % 2}")
                    if variant != "nodma":
                        if split:
                            # chunked so step 0 can start after ~1/4 of the
                            # data: k/q chunk c land before steps needing them
                            nc.sync.dma_start(kt[:, 0:W], k_d[p_, :, 0:W])
                            nc.sync.dma_start(qt[:, 0:W], q_d[p_, :, 0:W])
                            nc.sync.dma_start(va[:, :, :, :], v_d[p_, :, :, :, :])
                            for cc in range(1, NC_):
                                nc.sync.dma_start(
                                    kt[:, cc * W : (cc + 1) * W],
                                    k_d[p_, :, cc * W : (cc + 1) * W],
                                )
                                nc.sync.dma_start(
                                    qt[:, cc * W : (cc + 1) * W],
                                    q_d[p_, :, cc * W : (cc + 1) * W],
                                )
                        else:
                            nc.sync.dma_start(qt[:, :], q_d[p_, :, :])
                            nc.sync.dma_start(kt[:, :], k_d[p_, :, :])
                            nc.sync.dma_start(va[:, :, :, :], v_d[p_, :, :, :, :])
                    qt_p[p_], kt_p[p_], v_p[p_] = qt, kt, va

                load_pair(0, split=True)
                if NPAIR > 1:
                    load_pair(1)

                # flattened pipeline over both pairs
                D2 = 6  # mm2 lags mm1 by D2 steps
                total = NPAIR * NSTEP if variant != "empty" else 0
                sc_of = {}
                ex_of = {}
                ot_of = {}
                pending_p2 = []  # deferred DVE pass2s: (et, pt, ex, qrel0, diag)

                def emit_mask(pex, pq):
                    # zero ex where q < s within the diagonal block
                    nc.gpsimd.affine_select(
                        out=pex[:, :, pq : pq + P],
                        in_=pex[:, :, pq : pq + P],
                        compare_op=mybir.AluOpType.is_ge,
                        fill=0.0,
                        base=0,
                        channel_multiplier=-1,
                        pattern=[[0, 2], [1, P]],
                    )

                def flush_p2(upto_et):
                    while pending_p2 and pending_p2[0][0] <= upto_et:
                        _, ppt, pex, pq, pdiag = pending_p2.pop(0)
                        nc.vector._custom_dve(
                            exp_p2,
                            out=pex[:, :, pq:W],
                            in0=ppt[:, :, pq:W],
                            s0=float(np.exp(-EXP_BIAS)),
                        )
                        if pdiag:
                            emit_mask(pex, pq)

                def mm1_stage(it):
                    if it < total:
                        pi, si = divmod(it, NSTEP)
                        c, j, qrel0, diag, first, last = pair_steps[si]
                        qt, kt = qt_p[pi], kt_p[pi]
                        sc = ps_sc.tile([P, 2, W], f32, tag="sc")
                        for h in range(2):
                            nc.tensor.matmul(
                                sc[:, h, qrel0:W],
                                kt[h * 64 : h * 64 + 64, j * P : (j + 1) * P],
                                qt[h * 64 : h * 64 + 64, c * W + qrel0 : (c + 1) * W],
                                start=True,
                                stop=True,
                            )
                        sc_of[it] = sc

                def mm2_stage(it):
                    mt = it - D2
                    if 0 <= mt < total and variant not in ("nomm2", "mm1only"):
                        pi, si = divmod(mt, NSTEP)
                        c, j, qrel0, diag, first, last = pair_steps[si]
                        va = v_p[pi]
                        flush_p2(mt)
                        ex = ex_of.pop(mt) if variant != "noexp" else dummy_ex
                        for h in range(2):
                            hid = 2 * pi + h
                            if first:
                                ot_of[hid] = ps_ot.tile(
                                    [E + 1, W], f32, tag=f"ot{h}", name=f"ot{h}"
                                )
                            ot = ot_of[hid]
                            nc.tensor.matmul(
                                ot[:, qrel0:W],
                                va[:, h, j, :],
                                ex[:, h, qrel0:W],
                                start=first,
                                stop=last,
                                skip_group_check=True,
                            )
                            if not last:
                                continue
                            # evacuate OT: halves on ACT/DVE, then DMA
                            oc = oc_pool.tile([E + 1, W], f32, tag="oc")
                            bal["act"] += (W // 2 + 86) / 1.2
                            bal["dve"] += (W // 2 + 60) / 0.96
                            nc.scalar.activation(
                                oc[:, 0 : W // 2],
                                ot[:, 0 : W // 2],
                                mybir.ActivationFunctionType.Copy,
                            )
                            nc.vector.tensor_copy(
                                oc[:, W // 2 : W], ot[:, W // 2 : W]
                            )
                            nc.sync.dma_start(
                                o_d[hid, :, c * W : (c + 1) * W], oc[:, :]
                            )

                def exp_stage(it):
                    et = it - 1
                    if 0 <= et < total and variant not in ("noexp", "mm1only"):
                        pi, si = divmod(et, NSTEP)
                        c, j, qrel0, diag, first, last = pair_steps[si]
                        sc = sc_of.pop(et)
                        ex = ex_pool.tile([P, 2, W], f16, tag="ex")
                        width = 2 * (W - qrel0)
                        cost_a = (width + 111) / 1.2
                        cost_d = (2 * width + 120) / 0.96
                        eng = pick_engine(cost_a, cost_d)
                        if variant == "allact":
                            eng = "act"
                        if eng == "act":
                            nc.scalar.activation(
                                ex[:, :, qrel0:W],
                                sc[:, :, qrel0:W],
                                mybir.ActivationFunctionType.Exp,
                                scale=float(POW_N),
                                bias=bias_t[:, 0:1],
                            )
                        else:
                            # pass1 now (frees the PSUM sc slot promptly);
                            # pass2 deferred so back-to-back DVE tiles don't
                            # stall the score ring behind a 2-pass chain
                            pt = ex_pool.tile([P, 2, W], f32, tag="pt", bufs=4)
                            nc.vector._custom_dve(
                                exp_p1,
                                out=pt[:, :, qrel0:W],
                                in0=sc[:, :, qrel0:W],
                                s0=4.0,
                                s1=8.0,
                                imm2=float(64.0 / 6.0),
                            )
                            pending_p2.append((et, pt, ex, qrel0, diag))
                            flush_p2(et - 2)
                        if eng == "act" and diag:
                            emit_mask(ex, qrel0)
                        ex_of[et] = ex

                # batch-2: two steps' mm1s, then two mm2s, then two exps;
                # keeps the PE in one tiling mode for 4+ consecutive
                # matmuls, halving 64<->128 mode-switch drains
                BATCH = 5
                for it2 in range(0, total + D2, BATCH):
                    for o in range(BATCH):
                        mm1_stage(it2 + o)
                    for o in range(BATCH):
                        mm2_stage(it2 + o)
                    for o in range(BATCH):
                        exp_stage(it2 + o)
                flush_p2(total)

    nc.compile()
    return nc


def _get_nc():
    if "nc" not in _CACHE:
        _CACHE["nc"] = _build_nc()
    return _CACHE["nc"]


def _prep_in_maps(queries, keys, values):
    # [B, L, H, E] -> [B*H, E, L] f16; Q pre-scaled by SCALE/POW_N;
    # pairs stacked on the partition dim.
    qf = (
        np.transpose(queries, (0, 2, 3, 1)).reshape(B * H, E, L)
        * np.float32(SCALE / POW_N)
    ).astype(np.float16)
    kf = np.transpose(keys, (0, 2, 3, 1)).reshape(B * H, E, L).astype(np.float16)
    qf = qf.reshape(B * H // 2, P, L)
    kf = kf.reshape(B * H // 2, P, L)

    vf = np.transpose(values, (0, 2, 1, 3)).reshape(B * H, L, E).astype(np.float16)
    vf = np.concatenate([vf, np.ones((B * H, L, 1), np.float16)], axis=2)
    # [BH, L, 65] -> [BH, NT, P, 65] -> [BH, P, NT, 65]
    vf = vf.reshape(B * H, NT, P, E + 1).transpose(0, 2, 1, 3)
    # pair-stack: [BH/2, 2, P, NT, 65] -> [BH/2, P, 2, NT, 65]
    vf = vf.reshape(B * H // 2, 2, P, NT, E + 1).transpose(0, 2, 1, 3, 4)

    return [
        {
            "q": np.ascontiguousarray(qf[c * NPAIR : (c + 1) * NPAIR]),
            "k": np.ascontiguousarray(kf[c * NPAIR : (c + 1) * NPAIR]),
            "v": np.ascontiguousarray(vf[c * NPAIR : (c + 1) * NPAIR]),
        }
        for c in range(N_CORES)
    ]


def kernel(queries, keys, values):
    from concourse.bass_utils import run_bass_kernel_spmd

    nc = _get_nc()
    in_maps = _prep_in_maps(queries, keys, values)
    br = run_bass_kernel_spmd(nc, in_maps, core_ids=list(range(N_CORES)))
    ot = np.concatenate([r["o"] for r in br.results], axis=0)  # [BH, 65, L]
    out = ot[:, :E, :] / ot[:, E : E + 1, :]  # normalize
    out = out.reshape(B, H, E, L).transpose(0, 3, 1, 2)  # [B, L, H, E]
    return np.ascontiguousarray(out.astype(np.float32))


if __name__ == "__main__":
    rng = np.random.default_rng(0)
    q = rng.standard_normal((B, L, H, E)).astype(np.float32)
    k = rng.standard_normal((B, L, H, E)).astype(np.float32)
    v = rng.standard_normal((B, L, H, E)).astype(np.float32)
    out = kernel(queries=q, keys=k, values=v)
    print("out", out.shape, out.dtype)


# revision 51
# speedup vs baseline: 1.1668x; 1.1668x over previous
"""Causal full attention (B=2, L=2048, H=16, E=64) on 8 trn2 NeuronCores.

Sharding: 32 (b, h) head-slices split 4-per-core; each core runs the same
Bass program on its own slice (data parallel over batch*heads, per the
sharding hint), no cross-core comms.

Design (row-tiled QK^T, ACT+DVE split exp, host finalize):
  - Heads processed in PAIRS: Q^T/K^T stacked [128, L] f16 (head A rows
    0:64, head B rows 64:128). QK^T for the two heads runs as two
    concurrent row-tiled matmuls (tile_position (0,0)/(64,0) derived from
    base partitions), halving mm1 wall time vs contraction-64 alone.
  - Host pre-scales Q by scale/64, so PSUM scores arrive as s = x/64
    where x is the true scaled logit.
  - exp is the throughput wall (1 elem/lane/cycle on ScalarE), so it is
    split between ScalarE (exact exp(64*s - 2) via the free scale/bias)
    and a two-instruction custom VectorE pipeline: deg-4 Taylor of
    exp(x/16) then ^16 * e^-2 (the DVE fused-op budget is 8 ALU ops, so
    one instruction cannot reach exp accuracy; two passes match the
    ScalarE path to ~1e-4). Work is greedily load-balanced; the shared
    e^-2 factor cancels in the softmax division.
  - V is augmented with a ones column (65 rows); the P@V matmul then
    yields the softmax denominator for free in row 64.
  - Output O^T [65, 512] per (head, q-chunk) accumulates in PSUM over
    s-tiles, is copied to SBUF (halves split across ScalarE/VectorE),
    DMA'd out as [HPC, 65, L] f32; the host does denominator division and
    the [E, L] -> [L, E] transpose (cheap numpy, off the HW critical
    path).
  - PSUM: 3-ring of [128, 2, 512] score tiles (6 banks) + 2 per-head OT
    accumulators (2 banks) = all 8 banks.
  - Emission is batched 2-3 steps per stage so the PE stays in one tiling
    mode (64-row for QK^T vs 128 for PV) for several matmuls at a time -
    each mode switch drains the TensorE (~16us/iter when alternating
    every matmul).
  - DVE pass2 is emitted lazily so back-to-back DVE score tiles free the
    PSUM ring after pass1; OT evacuation copies are emitted ahead of exp
    calls on both engines so the next accumulation group is not stalled.
"""

import numpy as np

B, L, H, E = 2, 2048, 16, 64
N_CORES = 8
HPC = B * H // N_CORES   # head-slices per core = 4
NPAIR = HPC // 2         # head pairs per core = 2
P = 128
NT = L // P              # 16 s-tiles of 128
W = 512                  # q-chunk width (one PSUM bank)
NC_ = L // W             # 4 q-chunks
SCALE = 1.0 / np.sqrt(E)
EXP_BIAS = 2.0           # both engines compute exp(x - EXP_BIAS)
POW_N = 64.0             # host folds SCALE/POW_N into Q

_CACHE = {}


def _register_exp_ops():
    """Register a two-instruction VectorE exp pipeline (the DVE budget is 8
    ALU ops per fused instruction, so one instruction cannot reach exp
    accuracy over the score range):

      pass1: p = deg-4 Taylor of exp(x/16) in s = x/64:
             p = (((c*s + c)*s + b)*s + a)*s + 1, a=4, b=8, c=64/6
             (c3 == c4 == 64/6, so three scalar slots suffice) - 8 ops.
      pass2: out = (p^16) * e^-B via 4 squarings + one multiply - 5 ops.

    Combined: exp(x - B) to ~1e-4 relative over |x| <= 7, matching the
    ScalarE-exact path (both engines share the e^-B factor, which cancels
    in the softmax division)."""
    from concourse import dve_ops
    from concourse.dve_spec import (
        Spec,
        Src0,
        C0,
        C1,
        C2,
        One,
        sq,
        lower,
        _has_src1 as has_src1,
    )
    from concourse.dve_uop import DveOpSpec
    from concourse.dve_table_gen import dve_ver_for

    names = ("EXP_POLY16_ANT", "EXP_SQ4S_ANT")
    have = {op.name: op for op in dve_ops.OPS if op.name in names}
    if len(have) == 2:
        return have[names[0]], have[names[1]]

    body1 = (((C2 * Src0 + C2) * Src0 + C1) * Src0 + C0) * Src0 + One

    def ref1(in0, in1=None, s0=0.0, s1=0.0, imm2=0.0):
        s = in0.astype(np.float32)
        return (
            ((np.float32(imm2) * s + np.float32(imm2)) * s + np.float32(s1)) * s
            + np.float32(s0)
        ) * s + np.float32(1.0)

    b2 = Src0
    for _ in range(4):
        b2 = sq(b2)
    body2 = b2 * C0

    def ref2(in0, in1=None, s0=0.0, s1=0.0, imm2=0.0):
        r = in0.astype(np.float32)
        for _ in range(4):
            r = r * r
        return r * np.float32(s0)

    ver = dve_ver_for("TRN2")
    out = []
    for name, body, ref in (
        (names[0], body1, ref1),
        (names[1], body2, ref2),
    ):
        spec = Spec(body=body, reference=ref)
        row = dve_ops._CUSTOM_DVE_ROW_BASE + len(dve_ops.OPS)
        assert row < 0x20
        tmp = DveOpSpec(
            name=name, opcode=row, uops=lower(spec, ver=ver), rd1_en=has_src1(spec)
        )
        op = dve_ops.DveOp(name, spec, subdim=False, uops_sha={ver: tmp.sha(ver)})
        dve_ops.OPS.append(op)
        dve_ops.CUSTOM_DVE_SPECS[name] = spec
        dve_ops._SUB_OPCODE_FOR_NAME[name] = row
        out.append(op)
    return tuple(out)


def _build_nc(loop_iters=None, variant="full"):
    import concourse.bass as bass
    import concourse.tile as tile
    from concourse import mybir, bacc

    exp_p1, exp_p2 = _register_exp_ops()

    f32 = mybir.dt.float32
    f16 = mybir.dt.float16

    nc = bacc.Bacc("TRN2", target_bir_lowering=False, debug=False)

    q_d = nc.dram_tensor("q", [NPAIR, P, L], f16, kind="ExternalInput")
    k_d = nc.dram_tensor("k", [NPAIR, P, L], f16, kind="ExternalInput")
    v_d = nc.dram_tensor("v", [NPAIR, P, 2, NT, E + 1], f16, kind="ExternalInput")
    o_d = nc.dram_tensor("o", [HPC, E + 1, L], f32, kind="ExternalOutput")

    # per-(c,j) step list for one pair: q-chunk c covers q in [cW, (c+1)W);
    # s-tile j valid when j*P < (c+1)*W; within the chunk the valid q start
    # is qrel0 = max(0, j*P - c*W); diag when the 128-wide diagonal block
    # lands inside this chunk.
    pair_steps = []
    for c in range(NC_):
        jmax = ((c + 1) * W) // P  # exclusive
        for j in range(jmax):
            qrel0 = max(0, j * P - c * W)
            diag = j * P >= c * W
            first = j == 0
            last = j == jmax - 1
            pair_steps.append((c, j, qrel0, diag, first, last))
    NSTEP = len(pair_steps)  # 40

    # exp/copy engine load balancer (ns-scale costs from the cost model)
    bal = {"act": 0.0, "dve": 0.0}

    def pick_engine(cost_act, cost_dve):
        if bal["act"] + cost_act <= bal["dve"] + cost_dve:
            bal["act"] += cost_act
            return "act"
        bal["dve"] += cost_dve
        return "dve"

    with tile.TileContext(nc) as tc:
        with (
            tc.tile_pool(name="const", bufs=1) as const_pool,
            tc.tile_pool(name="io", bufs=2) as io_pool,
            tc.tile_pool(name="ex", bufs=8) as ex_pool,
            tc.tile_pool(name="oc", bufs=4) as oc_pool,
            tc.tile_pool(name="ps_sc", bufs=3, space="PSUM") as ps_sc,
            tc.tile_pool(name="ps_ot", bufs=1, space="PSUM") as ps_ot,
        ):
            bias_t = const_pool.tile([P, 1], f32)
            nc.vector.memset(bias_t[:, :], float(-EXP_BIAS))
            dummy_ex = None
            if variant in ("noexp",):
                dummy_ex = const_pool.tile([P, 2, W], f16)
                nc.vector.memset(dummy_ex[:, :, :], 0.001)
            # warm the exp table set in the preamble so the ~2.7us
            # ACT_TABLE_LOAD doesn't land inside the hardware loop body
            warm_t = const_pool.tile([P, 1], f32)
            nc.scalar.activation(
                warm_t[:, :], bias_t[:, :], mybir.ActivationFunctionType.Exp
            )
            import contextlib

            loop_cm = (
                tc.For_i(0, loop_iters, 1) if loop_iters else contextlib.nullcontext()
            )
            with loop_cm:
                qt_p, kt_p, v_p = {}, {}, {}

                def load_pair(p_, split=False):
                    qt = io_pool.tile([P, L], f16, tag=f"qt{p_ % 2}")
                    kt = io_pool.tile([P, L], f16, tag=f"kt{p_ % 2}")
                    va = io_pool.tile([P, 2, NT, E + 1], f16, tag=f"va{p_ % 2}")
                    if variant != "nodma":
                        if split:
                            # chunked so step 0 can start after ~1/4 of the
                            # data: k/q chunk c land before steps needing them
                            nc.sync.dma_start(kt[:, 0:W], k_d[p_, :, 0:W])
                            nc.sync.dma_start(qt[:, 0:W], q_d[p_, :, 0:W])
                            nc.sync.dma_start(va[:, :, :, :], v_d[p_, :, :, :, :])
                            for cc in range(1, NC_):
                                nc.sync.dma_start(
                                    kt[:, cc * W : (cc + 1) * W],
                                    k_d[p_, :, cc * W : (cc + 1) * W],
                                )
                                nc.sync.dma_start(
                                    qt[:, cc * W : (cc + 1) * W],
                                    q_d[p_, :, cc * W : (cc + 1) * W],
                                )
                        else:
                            nc.sync.dma_start(qt[:, :], q_d[p_, :, :])
                            nc.sync.dma_start(kt[:, :], k_d[p_, :, :])
                            nc.sync.dma_start(va[:, :, :, :], v_d[p_, :, :, :, :])
                    qt_p[p_], kt_p[p_], v_p[p_] = qt, kt, va

                load_pair(0, split=True)
                if NPAIR > 1:
                    load_pair(1)

                # flattened pipeline over both pairs
                D2 = 4  # mm2 lags mm1 by D2 steps
                total = NPAIR * NSTEP if variant != "empty" else 0
                sc_of = {}
                ex_of = {}
                ot_of = {}
                pending_p2 = []  # deferred DVE pass2s: (et, pt, ex, qrel0, diag)

                def emit_mask(pex, pq):
                    # zero ex where q < s within the diagonal block
                    nc.gpsimd.affine_select(
                        out=pex[:, :, pq : pq + P],
                        in_=pex[:, :, pq : pq + P],
                        compare_op=mybir.AluOpType.is_ge,
                        fill=0.0,
                        base=0,
                        channel_multiplier=-1,
                        pattern=[[0, 2], [1, P]],
                    )

                def flush_p2(upto_et):
                    while pending_p2 and pending_p2[0][0] <= upto_et:
                        _, ppt, pex, pq, pdiag = pending_p2.pop(0)
                        nc.vector._custom_dve(
                            exp_p2,
                            out=pex[:, :, pq:W],
                            in0=ppt[:, :, pq:W],
                            s0=float(np.exp(-EXP_BIAS)),
                        )
                        if pdiag:
                            emit_mask(pex, pq)

                def mm1_stage(it):
                    if it < total:
                        pi, si = divmod(it, NSTEP)
                        c, j, qrel0, diag, first, last = pair_steps[si]
                        qt, kt = qt_p[pi], kt_p[pi]
                        sc = ps_sc.tile([P, 2, W], f32, tag="sc")
                        for h in range(2):
                            nc.tensor.matmul(
                                sc[:, h, qrel0:W],
                                kt[h * 64 : h * 64 + 64, j * P : (j + 1) * P],
                                qt[h * 64 : h * 64 + 64, c * W + qrel0 : (c + 1) * W],
                                start=True,
                                stop=True,
                            )
                        sc_of[it] = sc

                def mm2_stage(it):
                    mt = it - D2
                    if 0 <= mt < total and variant not in ("nomm2", "mm1only"):
                        pi, si = divmod(mt, NSTEP)
                        c, j, qrel0, diag, first, last = pair_steps[si]
                        va = v_p[pi]
                        flush_p2(mt)
                        ex = ex_of.pop(mt) if variant != "noexp" else dummy_ex
                        for h in range(2):
                            hid = 2 * pi + h
                            if first:
                                ot_of[hid] = ps_ot.tile(
                                    [E + 1, W], f32, tag=f"ot{h}", name=f"ot{h}"
                                )
                            ot = ot_of[hid]
                            nc.tensor.matmul(
                                ot[:, qrel0:W],
                                va[:, h, j, :],
                                ex[:, h, qrel0:W],
                                start=first,
                                stop=last,
                                skip_group_check=True,
                            )
                            if not last:
                                continue
                            # evacuate OT: halves on ACT/DVE, then DMA
                            oc = oc_pool.tile([E + 1, W], f32, tag="oc")
                            bal["act"] += (W // 2 + 86) / 1.2
                            bal["dve"] += (W // 2 + 60) / 0.96
                            nc.scalar.activation(
                                oc[:, 0 : W // 2],
                                ot[:, 0 : W // 2],
                                mybir.ActivationFunctionType.Copy,
                            )
                            nc.vector.tensor_copy(
                                oc[:, W // 2 : W], ot[:, W // 2 : W]
                            )
                            nc.sync.dma_start(
                                o_d[hid, :, c * W : (c + 1) * W], oc[:, :]
                            )

                def exp_stage(it):
                    et = it - 1
                    if 0 <= et < total and variant not in ("noexp", "mm1only"):
                        pi, si = divmod(et, NSTEP)
                        c, j, qrel0, diag, first, last = pair_steps[si]
                        sc = sc_of.pop(et)
                        ex = ex_pool.tile([P, 2, W], f16, tag="ex")
                        width = 2 * (W - qrel0)
                        cost_a = (width + 111) / 1.2
                        cost_d = (2 * width + 120) / 0.96
                        eng = pick_engine(cost_a, cost_d)
                        if variant == "allact":
                            eng = "act"
                        if eng == "act":
                            nc.scalar.activation(
                                ex[:, :, qrel0:W],
                                sc[:, :, qrel0:W],
                                mybir.ActivationFunctionType.Exp,
                                scale=float(POW_N),
                                bias=bias_t[:, 0:1],
                            )
                        else:
                            # pass1 now (frees the PSUM sc slot promptly);
                            # pass2 deferred so back-to-back DVE tiles don't
                            # stall the score ring behind a 2-pass chain
                            pt = ex_pool.tile([P, 2, W], f32, tag="pt", bufs=4)
                            nc.vector._custom_dve(
                                exp_p1,
                                out=pt[:, :, qrel0:W],
                                in0=sc[:, :, qrel0:W],
                                s0=4.0,
                                s1=8.0,
                                imm2=float(64.0 / 6.0),
                            )
                            pending_p2.append((et, pt, ex, qrel0, diag))
                            flush_p2(et - 2)
                        if eng == "act" and diag:
                            emit_mask(ex, qrel0)
                        ex_of[et] = ex

                # batch-2: two steps' mm1s, then two mm2s, then two exps;
                # keeps the PE in one tiling mode for 4+ consecutive
                # matmuls, halving 64<->128 mode-switch drains
                BATCH = 3
                for it2 in range(0, total + D2, BATCH):
                    for o in range(BATCH):
                        mm1_stage(it2 + o)
                    for o in range(BATCH):
                        mm2_stage(it2 + o)
                    for o in range(BATCH):
                        exp_stage(it2 + o)
                flush_p2(total)

    nc.compile()
    return nc


def _get_nc():
    if "nc" not in _CACHE:
        _CACHE["nc"] = _build_nc()
    return _CACHE["nc"]


def _prep_in_maps(queries, keys, values):
    # [B, L, H, E] -> [B*H, E, L] f16; Q pre-scaled by SCALE/POW_N;
    # pairs stacked on the partition dim.
    qf = (
        np.transpose(queries, (0, 2, 3, 1)).reshape(B * H, E, L)
        * np.float32(SCALE / POW_N)
    ).astype(np.float16)
    kf = np.transpose(keys, (0, 2, 3, 1)).reshape(B * H, E, L).astype(np.float16)
    qf = qf.reshape(B * H // 2, P, L)
    kf = kf.reshape(B * H // 2, P, L)

    vf = np.transpose(values, (0, 2, 1, 3)).reshape(B * H, L, E).astype(np.float16)
    vf = np.concatenate([vf, np.ones((B * H, L, 1), np.float16)], axis=2)
    # [BH, L, 65] -> [BH, NT, P, 65] -> [BH, P, NT, 65]
    vf = vf.reshape(B * H, NT, P, E + 1).transpose(0, 2, 1, 3)
    # pair-stack: [BH/2, 2, P, NT, 65] -> [BH/2, P, 2, NT, 65]
    vf = vf.reshape(B * H // 2, 2, P, NT, E + 1).transpose(0, 2, 1, 3, 4)

    return [
        {
            "q": np.ascontiguousarray(qf[c * NPAIR : (c + 1) * NPAIR]),
            "k": np.ascontiguousarray(kf[c * NPAIR : (c + 1) * NPAIR]),
            "v": np.ascontiguousarray(vf[c * NPAIR : (c + 1) * NPAIR]),
        }
        for c in range(N_CORES)
    ]


def kernel(queries, keys, values):
    from concourse.bass_utils import run_bass_kernel_spmd

    nc = _get_nc()
    in_maps = _prep_in_maps(queries, keys, values)
    br = run_bass_kernel_spmd(nc, in_maps, core_ids=list(range(N_CORES)))
    ot = np.concatenate([r["o"] for r in br.results], axis=0)  # [BH, 65, L]
    out = ot[:, :E, :] / ot[:, E : E + 1, :]  # normalize
    out = out.reshape(B, H, E, L).transpose(0, 3, 1, 2)  # [B, L, H, E]
    return np.ascontiguousarray(out.astype(np.float32))


if __name__ == "__main__":
    rng = np.random.default_rng(0)
    q = rng.standard_normal((B, L, H, E)).astype(np.float32)
    k = rng.standard_normal((B, L, H, E)).astype(np.float32)
    v = rng.standard_normal((B, L, H, E)).astype(np.float32)
    out = kernel(queries=q, keys=k, values=v)
    print("out", out.shape, out.dtype)


# revision 56
# speedup vs baseline: 1.1685x; 1.0014x over previous
"""Causal full attention (B=2, L=2048, H=16, E=64) on 8 trn2 NeuronCores.

Sharding: 32 (b, h) head-slices split 4-per-core; each core runs the same
Bass program on its own slice (data parallel over batch*heads, per the
sharding hint), no cross-core comms.

Design (row-tiled QK^T, ACT+DVE split exp, host finalize):
  - Heads processed in PAIRS: Q^T/K^T stacked [128, L] f16 (head A rows
    0:64, head B rows 64:128). QK^T for the two heads runs as two
    concurrent row-tiled matmuls (tile_position (0,0)/(64,0) derived from
    base partitions), halving mm1 wall time vs contraction-64 alone.
  - Host pre-scales Q by scale/64, so PSUM scores arrive as s = x/64
    where x is the true scaled logit.
  - exp is the throughput wall (1 elem/lane/cycle on ScalarE), so it is
    split between ScalarE (exact exp(64*s - 2) via the free scale/bias)
    and a two-instruction custom VectorE pipeline: deg-4 Taylor of
    exp(x/16) then ^16 * e^-2 (the DVE fused-op budget is 8 ALU ops, so
    one instruction cannot reach exp accuracy; two passes match the
    ScalarE path to ~1e-4). Work is greedily load-balanced; the shared
    e^-2 factor cancels in the softmax division.
  - V is augmented with a ones column (65 rows); the P@V matmul then
    yields the softmax denominator for free in row 64.
  - Output O^T [65, 512] per (head, q-chunk) accumulates in PSUM over
    s-tiles, is copied to SBUF (halves split across ScalarE/VectorE),
    DMA'd out as [HPC, 65, L] f32; the host does denominator division and
    the [E, L] -> [L, E] transpose (cheap numpy, off the HW critical
    path).
  - PSUM: 3-ring of [128, 2, 512] score tiles (6 banks) + 2 per-head OT
    accumulators (2 banks) = all 8 banks.
  - Emission is batched 3 steps per stage so the PE stays in one tiling
    mode (64-row for QK^T vs 128 for PV) for several matmuls at a time -
    each mode switch drains the TensorE (~16us/iter when alternating
    every matmul).
  - DVE pass2 is emitted lazily so back-to-back DVE score tiles free the
    PSUM ring after pass1; OT evacuation copies are emitted ahead of exp
    calls on both engines so the next accumulation group is not stalled.
"""

import numpy as np

B, L, H, E = 2, 2048, 16, 64
N_CORES = 8
HPC = B * H // N_CORES   # head-slices per core = 4
NPAIR = HPC // 2         # head pairs per core = 2
P = 128
NT = L // P              # 16 s-tiles of 128
W = 512                  # q-chunk width (one PSUM bank)
NC_ = L // W             # 4 q-chunks
SCALE = 1.0 / np.sqrt(E)
EXP_BIAS = 2.0           # both engines compute exp(x - EXP_BIAS)
POW_N = 64.0             # host folds SCALE/POW_N into Q

_CACHE = {}


def _register_exp_ops():
    """Register a two-instruction VectorE exp pipeline (the DVE budget is 8
    ALU ops per fused instruction, so one instruction cannot reach exp
    accuracy over the score range):

      pass1: p = deg-4 Taylor of exp(x/16) in s = x/64:
             p = (((c*s + c)*s + b)*s + a)*s + 1, a=4, b=8, c=64/6
             (c3 == c4 == 64/6, so three scalar slots suffice) - 8 ops.
      pass2: out = (p^16) * e^-B via 4 squarings + one multiply - 5 ops.

    Combined: exp(x - B) to ~1e-4 relative over |x| <= 7, matching the
    ScalarE-exact path (both engines share the e^-B factor, which cancels
    in the softmax division)."""
    from concourse import dve_ops
    from concourse.dve_spec import (
        Spec,
        Src0,
        C0,
        C1,
        C2,
        One,
        sq,
        lower,
        _has_src1 as has_src1,
    )
    from concourse.dve_uop import DveOpSpec
    from concourse.dve_table_gen import dve_ver_for

    names = ("EXP_POLY16_ANT", "EXP_SQ4S_ANT")
    have = {op.name: op for op in dve_ops.OPS if op.name in names}
    if len(have) == 2:
        return have[names[0]], have[names[1]]

    body1 = (((C2 * Src0 + C2) * Src0 + C1) * Src0 + C0) * Src0 + One

    def ref1(in0, in1=None, s0=0.0, s1=0.0, imm2=0.0):
        s = in0.astype(np.float32)
        return (
            ((np.float32(imm2) * s + np.float32(imm2)) * s + np.float32(s1)) * s
            + np.float32(s0)
        ) * s + np.float32(1.0)

    b2 = Src0
    for _ in range(4):
        b2 = sq(b2)
    body2 = b2 * C0

    def ref2(in0, in1=None, s0=0.0, s1=0.0, imm2=0.0):
        r = in0.astype(np.float32)
        for _ in range(4):
            r = r * r
        return r * np.float32(s0)

    ver = dve_ver_for("TRN2")
    out = []
    for name, body, ref in (
        (names[0], body1, ref1),
        (names[1], body2, ref2),
    ):
        spec = Spec(body=body, reference=ref)
        row = dve_ops._CUSTOM_DVE_ROW_BASE + len(dve_ops.OPS)
        assert row < 0x20
        tmp = DveOpSpec(
            name=name, opcode=row, uops=lower(spec, ver=ver), rd1_en=has_src1(spec)
        )
        op = dve_ops.DveOp(name, spec, subdim=False, uops_sha={ver: tmp.sha(ver)})
        dve_ops.OPS.append(op)
        dve_ops.CUSTOM_DVE_SPECS[name] = spec
        dve_ops._SUB_OPCODE_FOR_NAME[name] = row
        out.append(op)
    return tuple(out)


def _build_nc(loop_iters=None, variant="full"):
    import concourse.bass as bass
    import concourse.tile as tile
    from concourse import mybir, bacc

    exp_p1, exp_p2 = _register_exp_ops()

    f32 = mybir.dt.float32
    f16 = mybir.dt.float16

    nc = bacc.Bacc("TRN2", target_bir_lowering=False, debug=False)

    q_d = nc.dram_tensor("q", [NPAIR, P, L], f16, kind="ExternalInput")
    k_d = nc.dram_tensor("k", [NPAIR, P, L], f16, kind="ExternalInput")
    v_d = nc.dram_tensor("v", [NPAIR, P, 2, NT, E + 1], f16, kind="ExternalInput")
    o_d = nc.dram_tensor("o", [HPC, E + 1, L], f32, kind="ExternalOutput")

    # per-(c,j) step list for one pair: q-chunk c covers q in [cW, (c+1)W);
    # s-tile j valid when j*P < (c+1)*W; within the chunk the valid q start
    # is qrel0 = max(0, j*P - c*W); diag when the 128-wide diagonal block
    # lands inside this chunk.
    pair_steps = []
    for c in range(NC_):
        jmax = ((c + 1) * W) // P  # exclusive
        for j in range(jmax):
            qrel0 = max(0, j * P - c * W)
            diag = j * P >= c * W
            first = j == 0
            last = j == jmax - 1
            pair_steps.append((c, j, qrel0, diag, first, last))
    NSTEP = len(pair_steps)  # 40

    # exp/copy engine load balancer (ns-scale costs from the cost model)
    bal = {"act": 0.0, "dve": 0.0}

    def pick_engine(cost_act, cost_dve):
        if bal["act"] + cost_act <= bal["dve"] + cost_dve:
            bal["act"] += cost_act
            return "act"
        bal["dve"] += cost_dve
        return "dve"

    with tile.TileContext(nc) as tc:
        with (
            tc.tile_pool(name="const", bufs=1) as const_pool,
            tc.tile_pool(name="io", bufs=2) as io_pool,
            tc.tile_pool(name="ex", bufs=8) as ex_pool,
            tc.tile_pool(name="oc", bufs=4) as oc_pool,
            tc.tile_pool(name="ps_sc", bufs=3, space="PSUM") as ps_sc,
            tc.tile_pool(name="ps_ot", bufs=1, space="PSUM") as ps_ot,
        ):
            bias_t = const_pool.tile([P, 1], f32)
            nc.vector.memset(bias_t[:, :], float(-EXP_BIAS))
            dummy_ex = None
            if variant in ("noexp",):
                dummy_ex = const_pool.tile([P, 2, W], f16)
                nc.vector.memset(dummy_ex[:, :, :], 0.001)
            # warm the exp table set in the preamble so the ~2.7us
            # ACT_TABLE_LOAD doesn't land inside the hardware loop body
            warm_t = const_pool.tile([P, 1], f32)
            nc.scalar.activation(
                warm_t[:, :], bias_t[:, :], mybir.ActivationFunctionType.Exp
            )
            import contextlib

            loop_cm = (
                tc.For_i(0, loop_iters, 1) if loop_iters else contextlib.nullcontext()
            )
            with loop_cm:
                qt_p, kt_p, v_p = {}, {}, {}

                def load_pair(p_, split=False):
                    qt = io_pool.tile([P, L], f16, tag=f"qt{p_ % 2}")
                    kt = io_pool.tile([P, L], f16, tag=f"kt{p_ % 2}")
                    va = io_pool.tile([P, 2, NT, E + 1], f16, tag=f"va{p_ % 2}")
                    if variant != "nodma":
                        if split:
                            # chunked so step 0 can start after ~1/4 of the
                            # data: k/q chunk c land before steps needing them
                            nc.sync.dma_start(kt[:, 0:W], k_d[p_, :, 0:W])
                            nc.sync.dma_start(qt[:, 0:W], q_d[p_, :, 0:W])
                            nc.sync.dma_start(va[:, :, :, :], v_d[p_, :, :, :, :])
                            for cc in range(1, NC_):
                                nc.sync.dma_start(
                                    kt[:, cc * W : (cc + 1) * W],
                                    k_d[p_, :, cc * W : (cc + 1) * W],
                                )
                                nc.sync.dma_start(
                                    qt[:, cc * W : (cc + 1) * W],
                                    q_d[p_, :, cc * W : (cc + 1) * W],
                                )
                        else:
                            nc.sync.dma_start(qt[:, :], q_d[p_, :, :])
                            nc.sync.dma_start(kt[:, :], k_d[p_, :, :])
                            nc.sync.dma_start(va[:, :, :, :], v_d[p_, :, :, :, :])
                    qt_p[p_], kt_p[p_], v_p[p_] = qt, kt, va

                load_pair(0, split=True)
                if NPAIR > 1:
                    load_pair(1)

                # flattened pipeline over both pairs
                D2 = 4  # mm2 lags mm1 by D2 steps
                total = NPAIR * NSTEP if variant != "empty" else 0
                sc_of = {}
                ex_of = {}
                ot_of = {}
                pending_p2 = []  # deferred DVE pass2s: (et, pt, ex, qrel0, diag)

                def emit_mask(pex, pq):
                    # zero ex where q < s within the diagonal block
                    nc.gpsimd.affine_select(
                        out=pex[:, :, pq : pq + P],
                        in_=pex[:, :, pq : pq + P],
                        compare_op=mybir.AluOpType.is_ge,
                        fill=0.0,
                        base=0,
                        channel_multiplier=-1,
                        pattern=[[0, 2], [1, P]],
                    )

                def flush_p2(upto_et):
                    while pending_p2 and pending_p2[0][0] <= upto_et:
                        _, ppt, pex, pq, pdiag = pending_p2.pop(0)
                        nc.vector._custom_dve(
                            exp_p2,
                            out=pex[:, :, pq:W],
                            in0=ppt[:, :, pq:W],
                            s0=float(np.exp(-EXP_BIAS)),
                        )
                        if pdiag:
                            emit_mask(pex, pq)

                def mm1_stage(it):
                    if it < total:
                        pi, si = divmod(it, NSTEP)
                        c, j, qrel0, diag, first, last = pair_steps[si]
                        qt, kt = qt_p[pi], kt_p[pi]
                        sc = ps_sc.tile([P, 2, W], f32, tag="sc")
                        for h in range(2):
                            nc.tensor.matmul(
                                sc[:, h, qrel0:W],
                                kt[h * 64 : h * 64 + 64, j * P : (j + 1) * P],
                                qt[h * 64 : h * 64 + 64, c * W + qrel0 : (c + 1) * W],
                                start=True,
                                stop=True,
                            )
                        sc_of[it] = sc

                def mm2_stage(it):
                    mt = it - D2
                    if 0 <= mt < total and variant not in ("nomm2", "mm1only"):
                        pi, si = divmod(mt, NSTEP)
                        c, j, qrel0, diag, first, last = pair_steps[si]
                        va = v_p[pi]
                        flush_p2(mt)
                        ex = ex_of.pop(mt) if variant != "noexp" else dummy_ex
                        for h in range(2):
                            hid = 2 * pi + h
                            if first:
                                ot_of[hid] = ps_ot.tile(
                                    [E + 1, W], f32, tag=f"ot{h}", name=f"ot{h}"
                                )
                            ot = ot_of[hid]
                            nc.tensor.matmul(
                                ot[:, qrel0:W],
                                va[:, h, j, :],
                                ex[:, h, qrel0:W],
                                start=first,
                                stop=last,
                                skip_group_check=True,
                            )
                            if not last:
                                continue
                            # evacuate OT: halves on ACT/DVE, then DMA
                            oc = oc_pool.tile([E + 1, W], f32, tag="oc")
                            bal["act"] += (W // 2 + 86) / 1.2
                            bal["dve"] += (W // 2 + 60) / 0.96
                            nc.scalar.activation(
                                oc[:, 0 : W // 2],
                                ot[:, 0 : W // 2],
                                mybir.ActivationFunctionType.Copy,
                            )
                            nc.vector.tensor_copy(
                                oc[:, W // 2 : W], ot[:, W // 2 : W]
                            )
                            nc.sync.dma_start(
                                o_d[hid, :, c * W : (c + 1) * W], oc[:, :]
                            )

                def exp_stage(it):
                    et = it - 1
                    if 0 <= et < total and variant not in ("noexp", "mm1only"):
                        pi, si = divmod(et, NSTEP)
                        c, j, qrel0, diag, first, last = pair_steps[si]
                        sc = sc_of.pop(et)
                        ex = ex_pool.tile([P, 2, W], f16, tag="ex")
                        width = 2 * (W - qrel0)
                        cost_a = (width + 111) / 1.2
                        cost_d = (2 * width + 120) / 0.96
                        eng = pick_engine(cost_a, cost_d)
                        if variant == "allact":
                            eng = "act"
                        if eng == "act":
                            nc.scalar.activation(
                                ex[:, :, qrel0:W],
                                sc[:, :, qrel0:W],
                                mybir.ActivationFunctionType.Exp,
                                scale=float(POW_N),
                                bias=bias_t[:, 0:1],
                            )
                        else:
                            # pass1 now (frees the PSUM sc slot promptly);
                            # pass2 deferred so back-to-back DVE tiles don't
                            # stall the score ring behind a 2-pass chain
                            pt = ex_pool.tile([P, 2, W], f32, tag="pt", bufs=4)
                            nc.vector._custom_dve(
                                exp_p1,
                                out=pt[:, :, qrel0:W],
                                in0=sc[:, :, qrel0:W],
                                s0=4.0,
                                s1=8.0,
                                imm2=float(64.0 / 6.0),
                            )
                            pending_p2.append((et, pt, ex, qrel0, diag))
                            flush_p2(et - 2)
                        if eng == "act" and diag:
                            emit_mask(ex, qrel0)
                        ex_of[et] = ex

                # batch-2: two steps' mm1s, then two mm2s, then two exps;
                # keeps the PE in one tiling mode for 4+ consecutive
                # matmuls, halving 64<->128 mode-switch drains
                BATCH = 3
                for it2 in range(0, total + D2, BATCH):
                    for o in range(BATCH):
                        mm1_stage(it2 + o)
                    for o in range(BATCH):
                        mm2_stage(it2 + o)
                    for o in range(BATCH):
                        exp_stage(it2 + o)
                flush_p2(total)

    nc.compile()
    return nc


def _get_nc():
    if "nc" not in _CACHE:
        _CACHE["nc"] = _build_nc()
    return _CACHE["nc"]


def _prep_in_maps(queries, keys, values):
    # [B, L, H, E] -> [B*H, E, L] f16; Q pre-scaled by SCALE/POW_N;
    # pairs stacked on the partition dim.
    qf = (
        np.transpose(queries, (0, 2, 3, 1)).reshape(B * H, E, L)
        * np.float32(SCALE / POW_N)
    ).astype(np.float16)
    kf = np.transpose(keys, (0, 2, 3, 1)).reshape(B * H, E, L).astype(np.float16)
    qf = qf.reshape(B * H // 2, P, L)
    kf = kf.reshape(B * H // 2, P, L)

    vf = np.transpose(values, (0, 2, 1, 3)).reshape(B * H, L, E).astype(np.float16)
    vf = np.concatenate([vf, np.ones((B * H, L, 1), np.float16)], axis=2)
    # [BH, L, 65] -> [BH, NT, P, 65] -> [BH, P, NT, 65]
    vf = vf.reshape(B * H, NT, P, E + 1).transpose(0, 2, 1, 3)
    # pair-stack: [BH/2, 2, P, NT, 65] -> [BH/2, P, 2, NT, 65]
    vf = vf.reshape(B * H // 2, 2, P, NT, E + 1).transpose(0, 2, 1, 3, 4)

    return [
        {
            "q": np.ascontiguousarray(qf[c * NPAIR : (c + 1) * NPAIR]),
            "k": np.ascontiguousarray(kf[c * NPAIR : (c + 1) * NPAIR]),
            "v": np.ascontiguousarray(vf[c * NPAIR : (c + 1) * NPAIR]),
        }
        for c in range(N_CORES)
    ]


def kernel(queries, keys, values):
    from concourse.bass_utils import run_bass_kernel_spmd

    nc = _get_nc()
    in_maps = _prep_in_maps(queries, keys, values)
    br = run_bass_kernel_spmd(nc, in_maps, core_ids=list(range(N_CORES)))
    ot = np.concatenate([r["o"] for r in br.results], axis=0)  # [BH, 65, L]
    out = ot[:, :E, :] / ot[:, E : E + 1, :]  # normalize
    out = out.reshape(B, H, E, L).transpose(0, 3, 1, 2)  # [B, L, H, E]
    return np.ascontiguousarray(out.astype(np.float32))


if __name__ == "__main__":
    rng = np.random.default_rng(0)
    q = rng.standard_normal((B, L, H, E)).astype(np.float32)
    k = rng.standard_normal((B, L, H, E)).astype(np.float32)
    v = rng.standard_normal((B, L, H, E)).astype(np.float32)
    out = kernel(queries=q, keys=k, values=v)
    print("out", out.shape, out.dtype)
